# revision 59
# baseline (speedup 1.0000x reference)
"""Trainium2 Bass kernel for a 4-layer Mamba (selective SSM) event-denoising stack.

Model (per reference):
  x = features @ emb_W + emb_b                       [B, L, 128]
  4x mamba layers (d_inner=256, d_state=16, d_conv=4, dt_rank=8)
  out = sigmoid(x @ head_W + head_b)                 [B, L, 1]

Sharding over 8 NeuronCores: data-parallel over batch (4) x tensor-parallel
over d_inner (2).  Core c owns batch c//2 and d_inner half c%2.  Per layer,
two pairwise AllReduces: the x-projection partial [40, L] and the out-proj
partial [128, L].

Per-core dataflow ([partition=channel, free=time], float32r matmuls):
  - in_proj + causal depthwise conv fused into 4 PSUM-accumulating matmuls
    with host-precomputed lhsT_k = in_W_xi * conv_w[:, k] on a 3-column
    haloed x chunk.
  - B/C rows for all 16 states are partition-broadcast in ONE SWDGE DMA per
    chunk from a bf16 copy of x_dbl in DRAM, so du = dtx*B and hC = h*C run
    as bf16 tensor_tensor at the DVE 2x mode.  The scan
    h_t = dA_t*h_{t-1} + du_t is tensor_tensor_scan (fp32 state), chained
    across chunks via per-state h tiles, split between GpSimdE and VectorE.
    dA_s = exp(A[:, s]*dt) on ScalarE with per-partition scale.
  - DMA issue is spread across the SP and ACT HWDGE queues and the Pool
    SWDGE queue: the sequencer-side ~1.2us per dma_start is otherwise the
    top serial resource.
"""

import sys

sys.path.insert(0, "/opt/trn_rl_repo")

import numpy as np

N_LAYERS = 4
D_MODEL = 128
D_STATE = 16
D_CONV = 4
D_INNER = 256
DT_RANK = 8
FEAT = 16
BATCH = 4
SEQ = 8192

N_CORES = 8
D_OWN = D_INNER // 2  # 128 channels per core

# activation-table ids for gen3 (see hw_specs.get_activation_tables):
# 0=exp_and_others, 5=natural_log, 6=natural_log_exp_and_others (has both
# Exp and Ln), 18=silu_and_others, 2=sigmoid_and_others
_ACT_TABLE_MERGE = {0: 6, 5: 6}


def _fix_act_tables(nc):
    """Retarget Exp-only/Ln-only table loads to the table containing both,
    then drop consecutive duplicate loads.  The compiler picks the first
    table per function, which thrashes ACT with a 1.3us table load per
    chunk between Exp and Ln."""
    import concourse.mybir as mybir

    for b in nc.main_func.blocks:
        keep = []
        cur = None
        for ins in b.instructions:
            if isinstance(ins, mybir.InstLoadActFuncSet):
                tid = _ACT_TABLE_MERGE.get(ins.act_func_set_id, ins.act_func_set_id)
                ins.act_func_set_id = tid
                if tid == cur and not ins.has_wait() and not ins.has_update():
                    continue  # redundant reload of the live table
                cur = tid
            keep.append(ins)
        if len(keep) != len(b.instructions):
            b.instructions[:] = keep


def build_program(L=SEQ, T=512, debug_probes=False, no_cc=False, pool_hc=(1, 2, 3, 4, 5, 6, 7, 8), n_ar_chunks=4):
    """Build the SPMD Bass/Tile program (same program for all 8 cores)."""
    from contextlib import ExitStack

    import concourse.bass as bass
    import concourse.tile as tile
    from concourse import bacc, mybir

    DT = mybir.dt.float32
    BF = mybir.dt.bfloat16
    F32R = mybir.dt.float32r
    AF = mybir.ActivationFunctionType
    OP = mybir.AluOpType
    NCH = L // T       # chunks
    # the software-pipelined schedule needs NCH >= 2*(NCH//NQ): phase-A
    # chunks must lead phase B by at least one AR quarter
    n_ar_chunks = max(2, min(n_ar_chunks, NCH // 2))
    assert NCH >= 2 * (NCH // n_ar_chunks), (
        f"pipeline needs NCH >= 2*CPQ; NCH={NCH} NQ={n_ar_chunks}"
    )
    HALO = D_CONV - 1  # 3
    TE = 2048          # embed/head chunk
    NE = L // TE

    def f32r(ap):
        return ap.bitcast(F32R)

    nc = bacc.Bacc(
        "TRN2",
        target_bir_lowering=False,
        debug=False,
        enable_asserts=False,
        num_devices=N_CORES,
    )

    # ---- external inputs (per-core data; same names on every core) ----
    featT = nc.dram_tensor("featT", [FEAT, L], DT, kind="ExternalInput").ap()
    emb_w = nc.dram_tensor("emb_w", [FEAT, D_MODEL], DT, kind="ExternalInput").ap()
    emb_b = nc.dram_tensor("emb_b", [D_MODEL, 1], DT, kind="ExternalInput").ap()
    head_w = nc.dram_tensor("head_w", [D_MODEL, 1], DT, kind="ExternalInput").ap()
    head_b = nc.dram_tensor("head_b", [1, 1], DT, kind="ExternalInput").ap()

    lw = []  # per-layer weight dram APs
    for l in range(N_LAYERS):
        d = dict(
            w_z=nc.dram_tensor(f"w_z_{l}", [D_MODEL, D_OWN], DT, kind="ExternalInput").ap(),
            convb=nc.dram_tensor(f"convb_{l}", [D_OWN, 1], DT, kind="ExternalInput").ap(),
            w_xp=nc.dram_tensor(f"w_xp_{l}", [D_OWN, DT_RANK + 2 * D_STATE], DT, kind="ExternalInput").ap(),
            w_dt=nc.dram_tensor(f"w_dt_{l}", [DT_RANK, D_OWN], DT, kind="ExternalInput").ap(),
            dt_b=nc.dram_tensor(f"dt_b_{l}", [D_OWN, 1], DT, kind="ExternalInput").ap(),
            a_neg=nc.dram_tensor(f"a_neg_{l}", [D_OWN, D_STATE], DT, kind="ExternalInput").ap(),
            dp=nc.dram_tensor(f"dp_{l}", [D_OWN, 1], DT, kind="ExternalInput").ap(),
            w_out=nc.dram_tensor(f"w_out_{l}", [D_OWN, D_MODEL], BF, kind="ExternalInput").ap(),
        )
        for k in range(D_CONV):
            d[f"w_cxi{k}"] = nc.dram_tensor(f"w_cxi_{l}_{k}", [D_MODEL, D_OWN], DT, kind="ExternalInput").ap()
        lw.append(d)

    probs = nc.dram_tensor("probs", [1, L], DT, kind="ExternalOutput").ap()
    dbg = {}
    if debug_probes:
        dbg["x0"] = nc.dram_tensor("dbg_x0", [D_MODEL, L], DT, kind="ExternalOutput").ap()
        for l in range(N_LAYERS):
            dbg[f"x{l + 1}"] = nc.dram_tensor(f"dbg_x{l + 1}", [D_MODEL, L], DT, kind="ExternalOutput").ap()
        dbg["xc0"] = nc.dram_tensor("dbg_xc0", [D_OWN, L], DT, kind="ExternalOutput").ap()
        dbg["zs0"] = nc.dram_tensor("dbg_zs0", [D_OWN, L], DT, kind="ExternalOutput").ap()
        dbg["xdbl0"] = nc.dram_tensor("dbg_xdbl0", [DT_RANK + 2 * D_STATE, L], DT, kind="ExternalOutput").ap()
        dbg["dt0"] = nc.dram_tensor("dbg_dt0", [D_OWN, L], DT, kind="ExternalOutput").ap()
        dbg["y0"] = nc.dram_tensor("dbg_y0", [D_OWN, L], DT, kind="ExternalOutput").ap()

    groups = [[2 * b, 2 * b + 1] for b in range(BATCH)]

    with tile.TileContext(nc) as tc, ExitStack() as ctx:
        wpool = ctx.enter_context(tc.tile_pool(name="w", bufs=1))
        cpool = ctx.enter_context(tc.tile_pool(name="chunk", bufs=2))
        hpool = ctx.enter_context(tc.tile_pool(name="hp", bufs=2))
        spool = ctx.enter_context(tc.tile_pool(name="scan", bufs=4))
        bcpool = ctx.enter_context(tc.tile_pool(name="bc", bufs=2))
        pmm = ctx.enter_context(tc.tile_pool(name="pmm", bufs=4, space="PSUM"))
        dram = ctx.enter_context(tc.tile_pool(name="dram", bufs=1, space="DRAM"))

        def load_w(ap, shape, tag, dtype=None):
            t = wpool.tile(shape, dtype or DT, tag=tag)
            if dtype is None:
                nc.gpsimd.dma_start(f32r(t[:]), f32r(ap))
            else:
                nc.gpsimd.dma_start(t[:], ap)
            return t

        # ---- preload all weights to SBUF ----
        emb_w_sb = load_w(emb_w, [FEAT, D_MODEL], "emb_w")
        emb_b_sb = load_w(emb_b, [D_MODEL, 1], "emb_b")
        head_w_sb = load_w(head_w, [D_MODEL, 1], "head_w")
        head_b_sb = load_w(head_b, [1, 1], "head_b")
        lsb = []
        for l in range(N_LAYERS):
            d = dict(
                w_z=load_w(lw[l]["w_z"], [D_MODEL, D_OWN], f"w_z{l}"),
                convb=load_w(lw[l]["convb"], [D_OWN, 1], f"convb{l}"),
                w_xp=load_w(lw[l]["w_xp"], [D_OWN, DT_RANK + 2 * D_STATE], f"w_xp{l}"),
                w_dt=load_w(lw[l]["w_dt"], [DT_RANK, D_OWN], f"w_dt{l}"),
                dt_b=load_w(lw[l]["dt_b"], [D_OWN, 1], f"dt_b{l}"),
                a_neg=load_w(lw[l]["a_neg"], [D_OWN, D_STATE], f"a_neg{l}"),
                dp=load_w(lw[l]["dp"], [D_OWN, 1], f"dp{l}"),
                w_out=load_w(lw[l]["w_out"], [D_OWN, D_MODEL], f"w_out{l}", dtype=BF),
            )
            for k in range(D_CONV):
                d[f"w_cxi{k}"] = load_w(lw[l][f"w_cxi{k}"], [D_MODEL, D_OWN], f"w_cxi{l}_{k}")
            lsb.append(d)

        # DRAM intermediates (quarter-major so collectives see contiguous
        # blocks)
        NQ = n_ar_chunks
        CPQ = NCH // NQ
        LQ = L // NQ
        x_cur = dram.tile([NQ, D_MODEL, LQ], DT, tag="x0")
        zpad = wpool.tile([D_MODEL, HALO], DT, tag="zpad")
        nc.vector.memset(zpad[:], 0.0)

        # ---- embedding: x0 = emb_W.T @ featT  (+ emb_b) ----
        def emit_embed(c):
            sl = slice(c * T, (c + 1) * T)
            f_c = cpool.tile([FEAT, T], DT, tag="dtr")
            nc.sync.dma_start(f32r(f_c[:]), f32r(featT[:, sl]))
            x_ps = pmm.tile([D_MODEL, T], DT, tag="mm")
            nc.tensor.matmul(x_ps[:], f32r(emb_w_sb[:]), f32r(f_c[:]), start=True, stop=True)
            x_sb = cpool.tile([D_MODEL, T], DT, tag="o_sb")
            nc.scalar.activation(x_sb[:], x_ps[:], AF.Identity, bias=emb_b_sb[:, 0:1])
            nc.scalar.dma_start(x_cur[c // CPQ, :, (c % CPQ) * T : (c % CPQ + 1) * T], x_sb[:])

        # ---- layers (software-pipelined emission) ----
        # Engines run their instruction streams in order, so layer l+1's
        # phase A is interleaved into layer l's phase B (offset by one AR
        # quarter) to kill the inter-layer pipeline bubble.
        LT = []  # per-layer dram tensors + scan state
        for l in range(N_LAYERS):
            LT.append(dict(
                xz=dram.tile([D_OWN, 2, L], DT, tag=f"xz{l}", name=f"xz{l}"),
                xdbl_part=dram.tile([NQ, DT_RANK + 2 * D_STATE, LQ], DT, tag=f"xdblp{l}", name=f"xdblp{l}"),
                xdbl_full=dram.tile([NQ, DT_RANK + 2 * D_STATE, LQ], DT, tag=f"xdblf{l}", name=f"xdblf{l}"),
                bc_bf=dram.tile([2 * D_STATE, L], BF, tag=f"bcbf{l}", name=f"bcbf{l}"),
                out_part=dram.tile([NQ, D_MODEL, LQ], DT, tag=f"outp{l}", name=f"outp{l}"),
                x_next=dram.tile([NQ, D_MODEL, LQ], DT, tag=f"x{l + 1}", name=f"xn{l + 1}"),
                h_prev=[None] * D_STATE,
            ))
        x_src = [x_cur] + [LT[l]["x_next"] for l in range(N_LAYERS)]

        def emit_A(l, c):
            W = lsb[l]
            xin = x_src[l]
            sl = slice(c * T, (c + 1) * T)
            q, lc = c // CPQ, c % CPQ
            x_ext = cpool.tile([D_MODEL, T + HALO], DT, tag="x_ext")
            if lc == 0:
                if c == 0:
                    nc.sync.dma_start(f32r(x_ext[:, 0:HALO]), f32r(zpad[:]))
                else:
                    nc.sync.dma_start(f32r(x_ext[:, 0:HALO]), f32r(xin[q - 1, :, LQ - HALO : LQ]))
                nc.sync.dma_start(f32r(x_ext[:, HALO:]), f32r(xin[q, :, 0:T]))
            else:
                nc.sync.dma_start(f32r(x_ext[:]), f32r(xin[q, :, lc * T - HALO : (lc + 1) * T]))

            # conv(in_proj(x)) = sum_k (in_W_xi * conv_w[:,k]).T @ x[t-3+k]
            xc_ps = pmm.tile([D_OWN, T], DT, tag="mm")
            for k in range(D_CONV):
                nc.tensor.matmul(
                    xc_ps[:], f32r(W[f"w_cxi{k}"][:]), f32r(x_ext[:, k : k + T]),
                    start=(k == 0), stop=(k == D_CONV - 1),
                )
            xz_sb = cpool.tile([D_OWN, 2, T], DT, tag="xz_sb")
            nc.scalar.activation(f32r(xz_sb[:, 0, :]), xc_ps[:], AF.Silu, bias=W["convb"][:, 0:1])

            # xproj partial: [40, T] = w_xp.T @ xc
            xp_ps = pmm.tile([DT_RANK + 2 * D_STATE, T], DT, tag="mm")
            nc.tensor.matmul(xp_ps[:], f32r(W["w_xp"][:]), f32r(xz_sb[:, 0, :]), start=True, stop=True)
            xp_sb = cpool.tile([DT_RANK + 2 * D_STATE, T], DT, tag="xp_sb")
            nc.scalar.activation(xp_sb[:], xp_ps[:], AF.Copy)
            nc.scalar.dma_start(LT[l]["xdbl_part"][c // CPQ, :, (c % CPQ) * T : (c % CPQ + 1) * T], xp_sb[:])

            # z-gate: zs = silu(x @ w_z)
            z_ps = pmm.tile([D_OWN, T], DT, tag="mm")
            nc.tensor.matmul(z_ps[:], f32r(W["w_z"][:]), f32r(x_ext[:, HALO:]), start=True, stop=True)
            nc.scalar.activation(f32r(xz_sb[:, 1, :]), z_ps[:], AF.Silu)
            nc.scalar.dma_start(LT[l]["xz"][:, :, sl], xz_sb[:])

        def emit_ARx(l, q):
            qsl = slice(q * LQ, (q + 1) * LQ)
            if no_cc:
                nc.sync.dma_start(LT[l]["xdbl_full"][q], LT[l]["xdbl_part"][q])
            else:
                nc.gpsimd.collective_compute(
                    "AllReduce", OP.add, replica_groups=groups,
                    ins=[LT[l]["xdbl_part"][q].opt()],
                    outs=[LT[l]["xdbl_full"][q].opt()],
                )
            # bf16 copy of B/C rows for the DMA partition-broadcasts
            cvt_f = cpool.tile([2 * D_STATE, LQ], DT, tag="cvt_f")
            nc.sync.dma_start(cvt_f[:], LT[l]["xdbl_full"][q, DT_RANK:, :])
            cvt_b = cpool.tile([2 * D_STATE, LQ], BF, tag="cvt_b")
            nc.scalar.activation(cvt_b[:], cvt_f[:], AF.Copy)
            nc.scalar.dma_start(LT[l]["bc_bf"][:, qsl], cvt_b[:])

        def emit_ARout(l, q):
            if no_cc:
                nc.sync.dma_start(LT[l]["x_next"][q], LT[l]["out_part"][q])
            else:
                nc.gpsimd.collective_compute(
                    "AllReduce", OP.add, replica_groups=groups,
                    ins=[LT[l]["out_part"][q].opt()],
                    outs=[LT[l]["x_next"][q].opt()],
                )

        def emit_B(l, c):
            W = lsb[l]
            bc_bf = LT[l]["bc_bf"]
            h_prev = LT[l]["h_prev"]
            sl = slice(c * T, (c + 1) * T)
            xz_c = cpool.tile([D_OWN, 2, T], DT, tag="xz_c")
            nc.sync.dma_start(xz_c[:], LT[l]["xz"][:, :, sl])
            xc_c = xz_c[:, 0, :]
            zs_c = xz_c[:, 1, :]

            # all-state B and C broadcast tiles, one SWDGE DMA each
            ball = bcpool.tile([D_OWN, D_STATE, T], BF, tag="ball")
            nc.sync.dma_start(
                ball[:],
                bass.AP(tensor=bc_bf.tensor, offset=bc_bf.offset + c * T,
                        ap=[[0, D_OWN], [L, D_STATE], [1, T]]),
            )
            call = bcpool.tile([D_OWN, D_STATE, T], BF, tag="call")
            nc.sync.dma_start(
                call[:],
                bass.AP(tensor=bc_bf.tensor, offset=bc_bf.offset + D_STATE * L + c * T,
                        ap=[[0, D_OWN], [L, D_STATE], [1, T]]),
            )

            # dt = softplus(dt_raw + dt_b) = relu(u) + ln(1 + exp(-|u|))
            dtr_c = cpool.tile([DT_RANK, T], DT, tag="dtr")
            nc.sync.dma_start(f32r(dtr_c[:]), f32r(LT[l]["xdbl_full"][c // CPQ, 0:DT_RANK, (c % CPQ) * T : (c % CPQ + 1) * T]))
            dt_ps = pmm.tile([D_OWN, T], DT, tag="mm")
            nc.tensor.matmul(dt_ps[:], f32r(W["w_dt"][:]), f32r(dtr_c[:]), start=True, stop=True)
            dt_a = cpool.tile([D_OWN, T], DT, tag="dt_a")
            nc.scalar.activation(dt_a[:], dt_ps[:], AF.Abs, bias=W["dt_b"][:, 0:1])
            dt_r = cpool.tile([D_OWN, T], DT, tag="dt_r")
            nc.scalar.activation(dt_r[:], dt_ps[:], AF.Relu, bias=W["dt_b"][:, 0:1])
            dt_e = cpool.tile([D_OWN, T], DT, tag="dt_a")
            nc.scalar.activation(dt_e[:], dt_a[:], AF.Exp, scale=-1.0)
            dt_l = cpool.tile([D_OWN, T], DT, tag="dt_a")
            nc.scalar.activation(dt_l[:], dt_e[:], AF.Ln, bias=1.0)
            dt_c = cpool.tile([D_OWN, T], DT, tag="dt_c")
            nc.vector.tensor_add(dt_c[:], dt_r[:], dt_l[:])
            dtx = cpool.tile([D_OWN, T], BF, tag="dtx")
            nc.vector.tensor_mul(dtx[:], dt_c[:], xc_c)

            y_c = cpool.tile([D_OWN, T], BF, tag="y_c")
            y_p = cpool.tile([D_OWN, T], BF, tag="y_p")
            for s in range(D_STATE):
                du = spool.tile([D_OWN, T], BF, tag="du")
                nc.vector.tensor_mul(du[:], dtx[:], ball[:, s, :])
                dA = spool.tile([D_OWN, T], DT, tag="dA")
                nc.scalar.activation(dA[:], dt_c[:], AF.Exp, scale=W["a_neg"][:, s : s + 1])
                h = hpool.tile([D_OWN, T], BF, tag=f"h{s}")
                init = 0.0 if c == 0 else h_prev[s][:, T - 1 : T]
                nc.vector.tensor_tensor_scan(
                    h[:], dA[:], du[:], init, op0=OP.mult, op1=OP.add
                )
                h_prev[s] = h

                # the C-weighted accumulation: a few states go to GpSimd with
                # a separate accumulator to offload the vector engine
                if s in pool_hc:
                    if s == pool_hc[0]:
                        nc.gpsimd.tensor_mul(y_p[:], h[:], call[:, s, :])
                    else:
                        hcp = spool.tile([D_OWN, T], BF, tag="hcp")
                        nc.gpsimd.tensor_mul(hcp[:], h[:], call[:, s, :])
                        nc.gpsimd.tensor_add(y_p[:], y_p[:], hcp[:])
                elif s == 0:
                    nc.vector.tensor_mul(y_c[:], h[:], call[:, s, :])
                else:
                    hc = spool.tile([D_OWN, T], BF, tag="hc")
                    nc.vector.tensor_mul(hc[:], h[:], call[:, s, :])
                    nc.vector.tensor_add(y_c[:], y_c[:], hc[:])
            nc.vector.tensor_add(y_c[:], y_c[:], y_p[:])

            # y = y + xc * Dp ; y *= silu(z)
            y2 = cpool.tile([D_OWN, T], BF, tag="y2")
            nc.vector.scalar_tensor_tensor(
                y2[:], xc_c, W["dp"][:, 0:1], y_c[:], op0=OP.mult, op1=OP.add
            )
            nc.vector.tensor_mul(y2[:], y2[:], zs_c)
            if debug_probes and l == 0:
                y2f = cpool.tile([D_OWN, T], DT, tag="y2f")
                nc.vector.tensor_copy(y2f[:], y2[:])
                nc.sync.dma_start(dbg["y0"][:, sl], y2f[:])
                nc.sync.dma_start(dbg["dt0"][:, sl], dt_c[:])

            # out partial: [128, T] = w_out.T @ y
            o_ps = pmm.tile([D_MODEL, T], DT, tag="mm")
            nc.tensor.matmul(o_ps[:], W["w_out"][:], y2[:], start=True, stop=True)
            o_sb = cpool.tile([D_MODEL, T], DT, tag="o_sb")
            nc.scalar.activation(o_sb[:], o_ps[:], AF.Copy)
            nc.scalar.dma_start(LT[l]["out_part"][c // CPQ, :, (c % CPQ) * T : (c % CPQ + 1) * T], o_sb[:])

        def emit_head(c):
            sl = slice(c * T, (c + 1) * T)
            x_c = cpool.tile([D_MODEL, T], DT, tag="o_sb")
            nc.sync.dma_start(f32r(x_c[:]), f32r(x_src[N_LAYERS][c // CPQ, :, (c % CPQ) * T : (c % CPQ + 1) * T]))
            h_ps = pmm.tile([1, T], DT, tag="mm")
            nc.tensor.matmul(h_ps[:], f32r(head_w_sb[:]), f32r(x_c[:]), start=True, stop=True)
            p_sb = cpool.tile([1, T], DT, tag="xp_sb")
            nc.scalar.activation(p_sb[:], h_ps[:], AF.Sigmoid, bias=head_b_sb[:, 0:1])
            nc.scalar.dma_start(probs[:, sl], p_sb[:])

        # prologue: embedding interleaved with layer-0 phase A (chunks
        # 0..NCH-CPQ-1); each layer's last-quarter A chunks are emitted
        # inside its own B loop's first quarter (their inputs only become
        # ready then), keeping every sequencer stream stall-free.
        def emit_A_step(l, c2):
            emit_A(l, c2)
            if c2 % CPQ == CPQ - 1:
                emit_ARx(l, c2 // CPQ)

        for c in range(NCH):
            emit_embed(c)
            if 1 <= c <= NCH - CPQ:
                emit_A_step(0, c - 1)

        for l in range(N_LAYERS):
            for c in range(NCH):
                emit_B(l, c)
                if c % CPQ == CPQ - 1:
                    emit_ARout(l, c // CPQ)
                if c < CPQ:
                    # epilogue of THIS layer's phase A (last quarter)
                    emit_A_step(l, NCH - CPQ + c)
                elif l < N_LAYERS - 1:
                    emit_A_step(l + 1, c - CPQ)
                else:
                    emit_head(c - CPQ)
        for c in range(NCH - CPQ, NCH):
            emit_head(c)
            if debug_probes:
                for q in range(NQ):
                    nc.sync.dma_start(dbg[f"x{l + 1}"][:, q * LQ : (q + 1) * LQ], LT[l]["x_next"][q])
                if l == 0:
                    nc.sync.dma_start(dbg["xc0"], LT[l]["xz"][:, 0, :])
                    nc.sync.dma_start(dbg["zs0"], LT[l]["xz"][:, 1, :])
                    for q in range(NQ):
                        nc.sync.dma_start(dbg["xdbl0"][:, q * LQ : (q + 1) * LQ], LT[l]["xdbl_full"][q])

    nc.compile()
    _fix_act_tables(nc)
    return nc


def make_in_maps(inputs, L=SEQ):
    """Host-side sharding: slice the full inputs into 8 per-core input maps."""
    f32 = np.float32
    features = np.asarray(inputs["features"], f32)
    emb_W = np.asarray(inputs["emb_W"], f32)
    emb_b = np.asarray(inputs["emb_b"], f32)
    in_W = np.asarray(inputs["in_W"], f32)
    conv_w = np.asarray(inputs["conv_w"], f32)
    conv_b = np.asarray(inputs["conv_b"], f32)
    xproj_W = np.asarray(inputs["xproj_W"], f32)
    dt_W = np.asarray(inputs["dt_W"], f32)
    dt_b = np.asarray(inputs["dt_b"], f32)
    A_log = np.asarray(inputs["A_log"], f32)
    Dp = np.asarray(inputs["Dp"], f32)
    out_W = np.asarray(inputs["out_W"], f32)
    head_W = np.asarray(inputs["head_W"], f32)
    head_b = np.asarray(inputs["head_b"], f32)

    a_neg = -np.exp(A_log)  # [NL, D_INNER, D_STATE]

    in_maps = []
    for core in range(N_CORES):
        b, h = core // 2, core % 2
        dsl = slice(h * D_OWN, (h + 1) * D_OWN)
        m = {
            "featT": np.ascontiguousarray(features[b, :L].T),  # [16, L]
            "emb_w": emb_W,
            "emb_b": emb_b.reshape(D_MODEL, 1),
            "head_w": head_W,
            "head_b": head_b.reshape(1, 1),
        }
        for l in range(N_LAYERS):
            w_xi = in_W[l][:, dsl]  # [128, 128]
            cw = conv_w[l][dsl]     # [128, 4]
            for k in range(D_CONV):
                m[f"w_cxi_{l}_{k}"] = np.ascontiguousarray(w_xi * cw[:, k][None, :])
            m[f"w_z_{l}"] = np.ascontiguousarray(in_W[l][:, D_INNER:][:, dsl])
            m[f"convb_{l}"] = np.ascontiguousarray(conv_b[l][dsl].reshape(D_OWN, 1))
            m[f"w_xp_{l}"] = np.ascontiguousarray(xproj_W[l][dsl])
            m[f"w_dt_{l}"] = np.ascontiguousarray(dt_W[l][:, dsl])
            m[f"dt_b_{l}"] = np.ascontiguousarray(dt_b[l][dsl].reshape(D_OWN, 1))
            m[f"a_neg_{l}"] = np.ascontiguousarray(a_neg[l][dsl])
            m[f"dp_{l}"] = np.ascontiguousarray(Dp[l][dsl].reshape(D_OWN, 1))
            m[f"w_out_{l}"] = np.ascontiguousarray(out_W[l][dsl]).astype(_bf16())
        in_maps.append(m)
    return in_maps


def _bf16():
    import ml_dtypes

    return ml_dtypes.bfloat16


_CACHE = {}


def _get_program(L=SEQ, T=512):
    key = (L, T)
    if key not in _CACHE:
        _CACHE[key] = build_program(L, T)
    return _CACHE[key]


def run(inputs, L=SEQ, T=512, trace=False):
    from concourse.bass_utils import run_bass_kernel_spmd

    nc = _get_program(L, T)
    in_maps = make_in_maps(inputs, L)
    res = run_bass_kernel_spmd(nc, in_maps, list(range(N_CORES)), trace=trace)
    outs = np.stack(
        [res.results[2 * b]["probs"].reshape(L, 1) for b in range(BATCH)]
    )
    return outs.astype(np.float32), res


def kernel(**inputs) -> np.ndarray:
    out, _ = run(inputs)
    return out
